# revision 15
# baseline (speedup 1.0000x reference)
"""MoE (2-expert SwiGLU) Trainium2 kernel, 8-core SPMD.

Strategy: since the MLPs have no biases and silu(0) = 0, MLP(0) = 0, so each
token only needs the expert it is routed to.  The host gathers tokens by
expert (MoE dispatch), cores 0-3 process expert-0 tokens and cores 4-7
expert-1 tokens, each core running a dense SwiGLU MLP with its expert's
weights.  The host scatters per-core outputs back into the full [B, S, D]
output.  If an expert draws slightly more than its 4 cores' balanced share
(4096 tokens), the small overflow (<=64 tokens) is computed on the host so
every core runs the same C<=1024 token capacity with clean 512-wide matmul
tiles.

Device dataflow (per core, transposed so no on-chip transposes are needed):
  yT = Wd^T @ (silu(Wg^T @ xT) * (Wu^T @ xT))
Weights are the stationary matmul operand, token-columns the moving operand.
All matmuls are bf16 with fp32 PSUM accumulation.  The FF intermediate `h`
for all of a core's tokens stays resident in SBUF, so each weight byte is
DMA'd exactly once per core.

Schedule notes (from perfetto/NTFF analysis of the previous version):
  - Two HWDGE rings are used: the sync ring streams the stage-1 weights
    (the critical stream), the scalar ring carries the token activations and
    prefetches ALL stage-2 down-weights into SBUF during stage 1, so the
    stage-1 -> stage-2 transition has no DMA dependency.
  - The PE HAM clock-gate needs ~3.4us of sustained busy time to reach
    2.4 GHz.  A short run of garbage matmuls on a memset scratch tile keeps
    the PE busy (and warms it) while the first real DMAs land.
  - PSUM is fully double-buffered in stage 1 (2 tags x 4 banks), so the PE
    never waits on the Act/DVE drain of the previous iteration.
  - The final output tile is stored in small chunks on alternating rings so
    the post-last-matmul tail is short.
"""

import sys

for _p in ("/opt/trn_rl_repo", "/root/.axon_site/_ro/trn_rl_repo"):
    if _p not in sys.path:
        sys.path.append(_p)

import numpy as np
import ml_dtypes

BF16 = ml_dtypes.bfloat16

D_MODEL = 1024
D_FF = 4096
P = 128
KD = D_MODEL // P  # 8   k-tiles over d_model
MF = D_FF // P     # 32  tiles over d_ff
N_CORES = 8
CPE = 4            # cores per expert
MAX_SPILL = 64     # max tokens/expert computed on host to even out cores

_program_cache: dict[tuple, object] = {}


def _token_tiles(C: int) -> tuple:
    """Split C (a multiple of 8) into the fewest <=512 tiles, near-equal,
    each a multiple of 8."""
    NT = max(1, -(-C // 512))
    t = 8 * (-(-C // (NT * 8)))
    tiles = []
    left = C
    for _ in range(NT):
        s = min(t, left)
        tiles.append(s)
        left -= s
    assert left == 0 and all(0 < s <= 512 and s % 8 == 0 for s in tiles), (C, tiles)
    return tuple(tiles)


def _build_program(tiles: tuple):
    """Bass program for one core: x [D,C] -> y [D,C], C = sum(tiles) tokens."""
    import concourse.tile as tile
    from concourse import mybir, bacc

    C = sum(tiles)
    NT = len(tiles)
    offs = [sum(tiles[:i]) for i in range(NT)]
    TSMAX = max(tiles)
    f32 = mybir.dt.float32
    b16 = mybir.dt.bfloat16

    # Keep the full stage-2 weight set resident in SBUF when it fits next to
    # x/h (it always does for the balanced C<=1024 case).  Very imbalanced
    # routing (huge C) needs the lean streaming layout to fit SBUF.
    wd_resident = C <= 1280
    lean = C > 1536

    nc = bacc.Bacc()
    xT = nc.declare_dram_parameter("xT", [P, KD, C], b16, isOutput=False)
    # w1[mf, p, kd, gu, c] = (wg if gu==0 else wu)[kd*128 + p, mf*128 + c]
    w1 = nc.declare_dram_parameter("w1", [MF, P, KD, 2, P], b16, isOutput=False)
    # wdp[md, p, kf, c] = wd[kf*128 + p, md*128 + c]
    wdp = nc.declare_dram_parameter("wd", [KD, P, MF, P], b16, isOutput=False)
    yT = nc.declare_dram_parameter("yT", [KD, P, C], b16, isOutput=True)

    with tile.TileContext(nc) as tc:
        with (
            tc.tile_pool(name="xp", bufs=1) as xp,
            tc.tile_pool(name="hp", bufs=1) as hp,
            tc.tile_pool(name="wsp", bufs=1) as wsp,
            tc.tile_pool(name="w1p", bufs=(2 if lean else 3)) as w1p,
            tc.tile_pool(name="wdpool", bufs=(1 if wd_resident else 2)) as wdpool,
            tc.tile_pool(name="silp", bufs=(2 if lean else 4)) as silp,
            tc.tile_pool(name="yp", bufs=4) as yp,
        ):
            x_sb = xp.tile([P, KD, C], b16)
            h_sb = hp.tile([P, MF, C], b16)
            ws = wsp.tile([P, 256], b16)
            nc.vector.memset(ws[:], 0.0)

            # Startup staging.  Sync ring: x k-slice 0 interleaved with the
            # first weight tile in kd-growing chunks (the PE consumes
            # kd-major), then the rest of the w1 stream.  Scalar ring: the
            # remaining x k-slices (its first trigger is delayed ~1.3us by
            # the activation-table load).  The stage-2 wd prefetch is spread
            # over mid-stage-1 iterations so it does not steal SDMA
            # bandwidth from the startup-critical x/w1 transfers.
            wt0 = w1p.tile([P, KD, 2, P], b16, tag="wt", name="wt_0")
            nc.sync.dma_start(wt0[:, 0], w1[0, :, 0])
            nc.sync.dma_start(wt0[:, 1:4], w1[0, :, 1:4])
            nc.sync.dma_start(wt0[:, 4:], w1[0, :, 4:])
            xh = min(512, C)
            for kd in range(KD):
                if kd in (1, 2) and C > 512:
                    # Halved so the first token tile's slice is consumable
                    # before the full k-slice lands (the PE is right on the
                    # heels of the x stream here).
                    nc.scalar.dma_start(x_sb[:, kd, :xh], xT[:, kd, :xh])
                    nc.scalar.dma_start(x_sb[:, kd, xh:], xT[:, kd, xh:])
                else:
                    nc.scalar.dma_start(x_sb[:, kd], xT[:, kd])
            if wd_resident:
                wd_sb = wdpool.tile([P, KD, MF, P], b16, name="wd_all")

            # PSUM: psg/psu/psy tiles span NT banks; matmul t writes the
            # bank-aligned [512*t, 512*t + tiles[t]) slice.  One pool for
            # both stages (a pool boundary would stall stage 2 on the drain
            # of the last stage-1 iteration).
            PSW = 512 * NT
            uniform = all(s == 512 for s in tiles) and not lean
            ps_bufs = max(1, 8 // (2 * NT))
            with tc.tile_pool(name="ps1", bufs=ps_bufs, space="PSUM") as ps1:
                # Garbage matmuls on the memset scratch keep the PE busy (and
                # the HAM clock-gate warming) while the first DMAs land.  The
                # warm tile shares the psg tag, so its bank is recycled (with
                # a WAR dependency) by a later iteration.
                wm = ps1.tile([P, PSW], f32, tag="psg", name="warm")
                for i in range(19):
                    nc.tensor.matmul(
                        wm[:, :256], ws[:, :128], ws[:],
                        start=(i == 0), stop=(i == 18),
                    )
                for mf in range(MF):
                    if mf == 0:
                        wt = wt0
                    else:
                        wt = w1p.tile([P, KD, 2, P], b16, tag="wt",
                                      name=f"wt_{mf}")
                        # w1[1] rides the scalar ring FIFO behind the x
                        # chunks (in two halves, so mf=1's first k-tiles can
                        # start as soon as the first half lands) and doesn't
                        # steal HBM bandwidth from the startup-critical x
                        # stream; later tiles stream on the sync ring,
                        # throttled by the pool buffers.
                        if mf == 1:
                            nc.scalar.dma_start(wt[:, :4], w1[mf, :, :4])
                            nc.scalar.dma_start(wt[:, 4:], w1[mf, :, 4:])
                        else:
                            nc.sync.dma_start(wt[:], w1[mf])
                    if wd_resident and mf % 2 == 0 and 10 <= mf < 10 + 2 * KD:
                        nc.scalar.dma_start(wd_sb[:, (mf - 10) // 2],
                                            wdp[(mf - 10) // 2])
                    psg = ps1.tile([P, PSW], f32, tag="psg", name=f"psg_{mf}")
                    psu = ps1.tile([P, PSW], f32, tag="psu", name=f"psu_{mf}")
                    for kd in range(KD):
                        for gu in range(2):
                            ps = psg if gu == 0 else psu
                            for t in range(NT):
                                nc.tensor.matmul(
                                    ps[:, 512 * t:512 * t + tiles[t]],
                                    wt[:, kd, gu],
                                    x_sb[:, kd, offs[t]:offs[t] + tiles[t]],
                                    start=(kd == 0),
                                    stop=(kd == KD - 1),
                                )
                    if uniform:
                        sil = silp.tile([P, PSW], f32, tag="sil",
                                        name=f"sil_{mf}")
                        nc.scalar.activation(
                            sil[:], psg[:],
                            mybir.ActivationFunctionType.Silu,
                        )
                        nc.vector.tensor_mul(h_sb[:, mf], sil[:], psu[:])
                    else:
                        for t in range(NT):
                            sil = silp.tile([P, TSMAX], f32, tag="sil",
                                            name=f"sil_{mf}_{t}")
                            nc.scalar.activation(
                                sil[:, :tiles[t]],
                                psg[:, 512 * t:512 * t + tiles[t]],
                                mybir.ActivationFunctionType.Silu,
                            )
                            nc.vector.tensor_mul(
                                h_sb[:, mf, offs[t]:offs[t] + tiles[t]],
                                sil[:, :tiles[t]],
                                psu[:, 512 * t:512 * t + tiles[t]],
                            )

                # Stage 2: y = Wd^T h, laid out [d-part, C].  psy reuses the
                # psg tag so there is no pool boundary between the stages.
                for md in range(KD):
                    if wd_resident:
                        wdt = wd_sb[:, md]
                    else:
                        wdtile = wdpool.tile([P, MF, P], b16, tag="wds",
                                             name=f"wd_{md}")
                        nc.sync.dma_start(wdtile[:], wdp[md])
                        wdt = wdtile[:]
                    psy = ps1.tile([P, PSW], f32, tag="psg", name=f"psy_{md}")
                    for t in range(NT):
                        sz = tiles[t]
                        off = offs[t]
                        last = (md == KD - 1 and t == NT - 1)
                        if not last:
                            for kf in range(MF):
                                nc.tensor.matmul(
                                    psy[:, 512 * t:512 * t + sz],
                                    wdt[:, kf],
                                    h_sb[:, kf, off:off + sz],
                                    start=(kf == 0),
                                    stop=(kf == MF - 1),
                                )
                            # Copies run on the scalar engine: it is idle in
                            # stage 2, and the strict-FIFO DVE queue would put
                            # these in the PE's semaphore dependency path.
                            y_sb = yp.tile([P, TSMAX], b16, tag="y",
                                           name=f"y_{md}_{t}")
                            nc.scalar.copy(
                                y_sb[:, :sz], psy[:, 512 * t:512 * t + sz])
                            nc.sync.dma_start(yT[md, :, off:off + sz],
                                              y_sb[:, :sz])
                        else:
                            # The very last tile runs as two half-width
                            # accumulation groups in different PSUM banks, so
                            # the first half's store overlaps the second
                            # half's matmuls and the post-last-matmul tail is
                            # one short copy + DMA.
                            h1 = (sz // 2) // 8 * 8
                            psyb = ps1.tile([P, PSW], f32, tag="psu",
                                            name="psy_last")
                            for g, (a, b, pst, po) in enumerate(
                                    [(0, h1, psy, 512 * t),
                                     (h1, sz, psyb, 0)]):
                                for kf in range(MF):
                                    nc.tensor.matmul(
                                        pst[:, po + a:po + b],
                                        wdt[:, kf],
                                        h_sb[:, kf, off + a:off + b],
                                        start=(kf == 0),
                                        stop=(kf == MF - 1),
                                    )
                                y_sb = yp.tile([P, TSMAX], b16, tag="y",
                                               name=f"y_last_{g}")
                                nc.scalar.copy(y_sb[:, a:b],
                                               pst[:, po + a:po + b])
                                nc.sync.dma_start(yT[md, :, off + a:off + b],
                                                  y_sb[:, a:b])

    nc.compile()
    return nc


def _get_program(tiles: tuple):
    if tiles not in _program_cache:
        _program_cache[tiles] = _build_program(tiles)
    return _program_cache[tiles]


def _pack_w1(wg: np.ndarray, wu: np.ndarray) -> np.ndarray:
    """[D, F] x2 -> [MF, P, KD, 2, P] bf16, matching the kernel's layout."""
    # w1[mf, p, kd, gu, c] = w_gu[kd*128 + p, mf*128 + c]
    stack = np.stack([wg, wu], axis=0)            # [gu, D, F]
    r = stack.reshape(2, KD, P, MF, P)            # [gu, kd, p, mf, c]
    return np.ascontiguousarray(r.transpose(3, 2, 1, 0, 4)).astype(BF16)


def _pack_wd(wd: np.ndarray) -> np.ndarray:
    """[F, D] -> [KD, P, MF, P] bf16. wdp[md, p, kf, c] = wd[kf*128+p, md*128+c]"""
    r = wd.reshape(MF, P, KD, P)                  # [kf, p, md, c]
    return np.ascontiguousarray(r.transpose(2, 1, 0, 3)).astype(BF16)


def _host_mlp(x, wg, wu, wd):
    g = x @ wg
    u = x @ wu
    h = (g / (1.0 + np.exp(-g))) * u
    return h @ wd


def _run_device(in_maps, tiles):
    from concourse.bass_utils import run_bass_kernel_spmd

    nc = _get_program(tiles)
    res = run_bass_kernel_spmd(nc, in_maps, core_ids=list(range(N_CORES)))
    return [r["yT"] for r in res.results]


def kernel(hidden_states, routing_mask, wg0, wu0, wd0, wg1, wu1, wd1,
           _run=None):
    hidden_states = np.asarray(hidden_states, dtype=np.float32)
    routing_mask = np.asarray(routing_mask)
    B, S, D = hidden_states.shape
    NTOK = B * S
    x = hidden_states.reshape(NTOK, D)
    mask = routing_mask.reshape(NTOK)
    w_np = [(np.asarray(wg0, np.float32), np.asarray(wu0, np.float32),
             np.asarray(wd0, np.float32)),
            (np.asarray(wg1, np.float32), np.asarray(wu1, np.float32),
             np.asarray(wd1, np.float32))]

    idx = [np.nonzero(mask == e)[0] for e in (0, 1)]
    # If an expert slightly exceeds its balanced share (CPE*1024 tokens),
    # compute the small overflow on the host so device cores keep clean
    # 512-wide tiles.  Only do it when it actually shrinks C.
    balanced = CPE * 1024
    spill = [0, 0]
    for e in (0, 1):
        over = len(idx[e]) - balanced
        if 0 < over <= MAX_SPILL:
            spill[e] = over
    maxpc_ns = max((len(idx[0]) + CPE - 1) // CPE,
                   (len(idx[1]) + CPE - 1) // CPE, 1)
    maxpc_sp = max((len(idx[0]) - spill[0] + CPE - 1) // CPE,
                   (len(idx[1]) - spill[1] + CPE - 1) // CPE, 1)
    if 8 * (-(-maxpc_sp // 8)) >= 8 * (-(-maxpc_ns // 8)):
        spill = [0, 0]
    spill_ids = [idx[e][len(idx[e]) - spill[e]:] for e in (0, 1)]
    dev_ids = [idx[e][:len(idx[e]) - spill[e]] for e in (0, 1)]

    maxpc = max((len(dev_ids[0]) + CPE - 1) // CPE,
                (len(dev_ids[1]) + CPE - 1) // CPE, 1)
    C = max(8 * (-(-maxpc // 8)), 32)
    tiles = _token_tiles(C)

    w1_packed = [_pack_w1(w_np[0][0], w_np[0][1]),
                 _pack_w1(w_np[1][0], w_np[1][1])]
    wd_packed = [_pack_wd(w_np[0][2]), _pack_wd(w_np[1][2])]

    in_maps = []
    chunks = []  # (expert, token_indices) per core
    for core in range(N_CORES):
        e = core // CPE
        slot = core % CPE
        ids = dev_ids[e]
        # split ids into CPE nearly-equal chunks
        bounds = [(len(ids) * i) // CPE for i in range(CPE + 1)]
        ids_c = ids[bounds[slot]:bounds[slot + 1]]
        chunks.append((e, ids_c))

        xc = np.zeros((C, D), dtype=np.float32)
        xc[: len(ids_c)] = x[ids_c]
        # xT[p, kd, c] = xc[c, kd*128 + p]
        xT = np.ascontiguousarray(
            xc.reshape(C, KD, P).transpose(2, 1, 0)
        ).astype(BF16)
        in_maps.append({
            "xT": xT,
            "w1": w1_packed[e],
            "wd": wd_packed[e],
        })

    run = _run if _run is not None else _run_device
    outs = run(in_maps, tiles)

    y_full = np.zeros((NTOK, D), dtype=np.float32)
    for core in range(N_CORES):
        _, ids_c = chunks[core]
        if len(ids_c) == 0:
            continue
        yTc = np.asarray(outs[core]).astype(np.float32).reshape(D, C)
        y_full[ids_c] = yTc[:, : len(ids_c)].T
    for e in (0, 1):
        if len(spill_ids[e]):
            wg, wu, wd = w_np[e]
            y_full[spill_ids[e]] = _host_mlp(x[spill_ids[e]], wg, wu, wd)
    return y_full.reshape(B, S, D)


# revision 17
# speedup vs baseline: 1.0032x; 1.0032x over previous
"""MoE (2-expert SwiGLU) Trainium2 kernel, 8-core SPMD.

Strategy: since the MLPs have no biases and silu(0) = 0, MLP(0) = 0, so each
token only needs the expert it is routed to.  The host gathers tokens by
expert (MoE dispatch), cores 0-3 process expert-0 tokens and cores 4-7
expert-1 tokens, each core running a dense SwiGLU MLP with its expert's
weights.  The host scatters per-core outputs back into the full [B, S, D]
output.  If an expert draws slightly more than its 4 cores' balanced share
(4096 tokens), the small overflow (<=64 tokens) is computed on the host so
every core runs the same C<=1024 token capacity with clean 512-wide matmul
tiles.

Device dataflow (per core, transposed so no on-chip transposes are needed):
  yT = Wd^T @ (silu(Wg^T @ xT) * (Wu^T @ xT))
Weights are the stationary matmul operand, token-columns the moving operand.
All matmuls are bf16 with fp32 PSUM accumulation.  The FF intermediate `h`
for all of a core's tokens stays resident in SBUF, so each weight byte is
DMA'd exactly once per core.

Schedule notes (from perfetto/NTFF analysis of the previous version):
  - Two HWDGE rings are used: the sync ring streams the stage-1 weights
    (the critical stream), the scalar ring carries the token activations and
    prefetches ALL stage-2 down-weights into SBUF during stage 1, so the
    stage-1 -> stage-2 transition has no DMA dependency.
  - The PE HAM clock-gate needs ~3.4us of sustained busy time to reach
    2.4 GHz.  A short run of garbage matmuls on a memset scratch tile keeps
    the PE busy (and warms it) while the first real DMAs land.
  - PSUM is fully double-buffered in stage 1 (2 tags x 4 banks), so the PE
    never waits on the Act/DVE drain of the previous iteration.
  - The final output tile is stored in small chunks on alternating rings so
    the post-last-matmul tail is short.
"""

import sys

for _p in ("/opt/trn_rl_repo", "/root/.axon_site/_ro/trn_rl_repo"):
    if _p not in sys.path:
        sys.path.append(_p)

import numpy as np
import ml_dtypes

BF16 = ml_dtypes.bfloat16

D_MODEL = 1024
D_FF = 4096
P = 128
KD = D_MODEL // P  # 8   k-tiles over d_model
MF = D_FF // P     # 32  tiles over d_ff
N_CORES = 8
CPE = 4            # cores per expert
MAX_SPILL = 64     # max tokens/expert computed on host to even out cores

_program_cache: dict[tuple, object] = {}


def _token_tiles(C: int) -> tuple:
    """Split C (a multiple of 8) into the fewest <=512 tiles, near-equal,
    each a multiple of 8."""
    NT = max(1, -(-C // 512))
    t = 8 * (-(-C // (NT * 8)))
    tiles = []
    left = C
    for _ in range(NT):
        s = min(t, left)
        tiles.append(s)
        left -= s
    assert left == 0 and all(0 < s <= 512 and s % 8 == 0 for s in tiles), (C, tiles)
    return tuple(tiles)


def _build_program(tiles: tuple):
    """Bass program for one core: x [D,C] -> y [D,C], C = sum(tiles) tokens."""
    import concourse.tile as tile
    from concourse import mybir, bacc

    C = sum(tiles)
    NT = len(tiles)
    offs = [sum(tiles[:i]) for i in range(NT)]
    TSMAX = max(tiles)
    f32 = mybir.dt.float32
    b16 = mybir.dt.bfloat16

    # Keep the full stage-2 weight set resident in SBUF when it fits next to
    # x/h (it always does for the balanced C<=1024 case).  Very imbalanced
    # routing (huge C) needs the lean streaming layout to fit SBUF.
    wd_resident = C <= 1280
    lean = C > 1536

    nc = bacc.Bacc()
    xT = nc.declare_dram_parameter("xT", [P, KD, C], b16, isOutput=False)
    # w1[mf, p, kd, gu, c] = (wg if gu==0 else wu)[kd*128 + p, mf*128 + c]
    w1 = nc.declare_dram_parameter("w1", [MF, P, KD, 2, P], b16, isOutput=False)
    # wdp[md, p, kf, c] = wd[kf*128 + p, md*128 + c]
    wdp = nc.declare_dram_parameter("wd", [KD, P, MF, P], b16, isOutput=False)
    yT = nc.declare_dram_parameter("yT", [KD, P, C], b16, isOutput=True)

    with tile.TileContext(nc) as tc:
        with (
            tc.tile_pool(name="xp", bufs=1) as xp,
            tc.tile_pool(name="hp", bufs=1) as hp,
            tc.tile_pool(name="wsp", bufs=1) as wsp,
            tc.tile_pool(name="w1p", bufs=(2 if lean else 3)) as w1p,
            tc.tile_pool(name="wdpool", bufs=(1 if wd_resident else 2)) as wdpool,
            tc.tile_pool(name="silp", bufs=(2 if lean else 4)) as silp,
            tc.tile_pool(name="yp", bufs=4) as yp,
        ):
            x_sb = xp.tile([P, KD, C], b16)
            h_sb = hp.tile([P, MF, C], b16)
            ws = wsp.tile([P, 256], b16)
            nc.vector.memset(ws[:], 0.0)

            # Startup staging.  Sync ring: x k-slice 0 interleaved with the
            # first weight tile in kd-growing chunks (the PE consumes
            # kd-major), then the rest of the w1 stream.  Scalar ring: the
            # remaining x k-slices (its first trigger is delayed ~1.3us by
            # the activation-table load).  The stage-2 wd prefetch is spread
            # over mid-stage-1 iterations so it does not steal SDMA
            # bandwidth from the startup-critical x/w1 transfers.
            wt0 = w1p.tile([P, KD, 2, P], b16, tag="wt", name="wt_0")
            nc.sync.dma_start(wt0[:, 0], w1[0, :, 0])
            nc.sync.dma_start(wt0[:, 1:4], w1[0, :, 1:4])
            nc.sync.dma_start(wt0[:, 4:], w1[0, :, 4:])
            for kd in range(KD):
                nc.scalar.dma_start(x_sb[:, kd], xT[:, kd])
            if wd_resident:
                wd_sb = wdpool.tile([P, KD, MF, P], b16, name="wd_all")

            # PSUM: psg/psu/psy tiles span NT banks; matmul t writes the
            # bank-aligned [512*t, 512*t + tiles[t]) slice.  One pool for
            # both stages (a pool boundary would stall stage 2 on the drain
            # of the last stage-1 iteration).
            PSW = 512 * NT
            uniform = all(s == 512 for s in tiles) and not lean
            ps_bufs = max(1, 8 // (2 * NT))
            with tc.tile_pool(name="ps1", bufs=ps_bufs, space="PSUM") as ps1:
                # Garbage matmuls on the memset scratch keep the PE busy (and
                # the HAM clock-gate warming) while the first DMAs land.  The
                # warm tile shares the psg tag, so its bank is recycled (with
                # a WAR dependency) by a later iteration.
                wm = ps1.tile([P, PSW], f32, tag="psg", name="warm")
                for i in range(18):
                    nc.tensor.matmul(
                        wm[:, :256], ws[:, :128], ws[:],
                        start=(i == 0), stop=(i == 17),
                    )
                for mf in range(MF):
                    if mf == 0:
                        wt = wt0
                    else:
                        wt = w1p.tile([P, KD, 2, P], b16, tag="wt",
                                      name=f"wt_{mf}")
                        # w1[1] rides the scalar ring FIFO behind the x
                        # chunks (in two halves, so mf=1's first k-tiles can
                        # start as soon as the first half lands) and doesn't
                        # steal HBM bandwidth from the startup-critical x
                        # stream; later tiles stream on the sync ring,
                        # throttled by the pool buffers.
                        if mf == 1:
                            nc.scalar.dma_start(wt[:, :4], w1[mf, :, :4])
                            nc.scalar.dma_start(wt[:, 4:], w1[mf, :, 4:])
                        else:
                            nc.sync.dma_start(wt[:], w1[mf])
                    if wd_resident and mf % 2 == 0 and 10 <= mf < 10 + 2 * KD:
                        nc.scalar.dma_start(wd_sb[:, (mf - 10) // 2],
                                            wdp[(mf - 10) // 2])
                    psg = ps1.tile([P, PSW], f32, tag="psg", name=f"psg_{mf}")
                    psu = ps1.tile([P, PSW], f32, tag="psu", name=f"psu_{mf}")
                    for kd in range(KD):
                        for gu in range(2):
                            ps = psg if gu == 0 else psu
                            for t in range(NT):
                                nc.tensor.matmul(
                                    ps[:, 512 * t:512 * t + tiles[t]],
                                    wt[:, kd, gu],
                                    x_sb[:, kd, offs[t]:offs[t] + tiles[t]],
                                    start=(kd == 0),
                                    stop=(kd == KD - 1),
                                )
                    if uniform:
                        sil = silp.tile([P, PSW], f32, tag="sil",
                                        name=f"sil_{mf}")
                        nc.scalar.activation(
                            sil[:], psg[:],
                            mybir.ActivationFunctionType.Silu,
                        )
                        nc.vector.tensor_mul(h_sb[:, mf], sil[:], psu[:])
                    else:
                        for t in range(NT):
                            sil = silp.tile([P, TSMAX], f32, tag="sil",
                                            name=f"sil_{mf}_{t}")
                            nc.scalar.activation(
                                sil[:, :tiles[t]],
                                psg[:, 512 * t:512 * t + tiles[t]],
                                mybir.ActivationFunctionType.Silu,
                            )
                            nc.vector.tensor_mul(
                                h_sb[:, mf, offs[t]:offs[t] + tiles[t]],
                                sil[:, :tiles[t]],
                                psu[:, 512 * t:512 * t + tiles[t]],
                            )

                # Stage 2: y = Wd^T h, laid out [d-part, C].  psy reuses the
                # psg tag so there is no pool boundary between the stages.
                for md in range(KD):
                    if wd_resident:
                        wdt = wd_sb[:, md]
                    else:
                        wdtile = wdpool.tile([P, MF, P], b16, tag="wds",
                                             name=f"wd_{md}")
                        nc.sync.dma_start(wdtile[:], wdp[md])
                        wdt = wdtile[:]
                    psy = ps1.tile([P, PSW], f32, tag="psg", name=f"psy_{md}")
                    for t in range(NT):
                        sz = tiles[t]
                        off = offs[t]
                        last = (md == KD - 1 and t == NT - 1)
                        if not last:
                            for kf in range(MF):
                                nc.tensor.matmul(
                                    psy[:, 512 * t:512 * t + sz],
                                    wdt[:, kf],
                                    h_sb[:, kf, off:off + sz],
                                    start=(kf == 0),
                                    stop=(kf == MF - 1),
                                )
                            # Copies run on the scalar engine: it is idle in
                            # stage 2, and the strict-FIFO DVE queue would put
                            # these in the PE's semaphore dependency path.
                            y_sb = yp.tile([P, TSMAX], b16, tag="y",
                                           name=f"y_{md}_{t}")
                            nc.scalar.copy(
                                y_sb[:, :sz], psy[:, 512 * t:512 * t + sz])
                            nc.sync.dma_start(yT[md, :, off:off + sz],
                                              y_sb[:, :sz])
                        else:
                            # The very last tile runs as two half-width
                            # accumulation groups in different PSUM banks, so
                            # the first half's store overlaps the second
                            # half's matmuls and the post-last-matmul tail is
                            # one short copy + DMA.
                            h1 = (sz // 2) // 8 * 8
                            psyb = ps1.tile([P, PSW], f32, tag="psu",
                                            name="psy_last")
                            for g, (a, b, pst, po) in enumerate(
                                    [(0, h1, psy, 512 * t),
                                     (h1, sz, psyb, 0)]):
                                for kf in range(MF):
                                    nc.tensor.matmul(
                                        pst[:, po + a:po + b],
                                        wdt[:, kf],
                                        h_sb[:, kf, off + a:off + b],
                                        start=(kf == 0),
                                        stop=(kf == MF - 1),
                                    )
                                y_sb = yp.tile([P, TSMAX], b16, tag="y",
                                               name=f"y_last_{g}")
                                nc.scalar.copy(y_sb[:, a:b],
                                               pst[:, po + a:po + b])
                                nc.sync.dma_start(yT[md, :, off + a:off + b],
                                                  y_sb[:, a:b])

    nc.compile()
    return nc


def _get_program(tiles: tuple):
    if tiles not in _program_cache:
        _program_cache[tiles] = _build_program(tiles)
    return _program_cache[tiles]


def _pack_w1(wg: np.ndarray, wu: np.ndarray) -> np.ndarray:
    """[D, F] x2 -> [MF, P, KD, 2, P] bf16, matching the kernel's layout."""
    # w1[mf, p, kd, gu, c] = w_gu[kd*128 + p, mf*128 + c]
    stack = np.stack([wg, wu], axis=0)            # [gu, D, F]
    r = stack.reshape(2, KD, P, MF, P)            # [gu, kd, p, mf, c]
    return np.ascontiguousarray(r.transpose(3, 2, 1, 0, 4)).astype(BF16)


def _pack_wd(wd: np.ndarray) -> np.ndarray:
    """[F, D] -> [KD, P, MF, P] bf16. wdp[md, p, kf, c] = wd[kf*128+p, md*128+c]"""
    r = wd.reshape(MF, P, KD, P)                  # [kf, p, md, c]
    return np.ascontiguousarray(r.transpose(2, 1, 0, 3)).astype(BF16)


def _host_mlp(x, wg, wu, wd):
    g = x @ wg
    u = x @ wu
    h = (g / (1.0 + np.exp(-g))) * u
    return h @ wd


def _run_device(in_maps, tiles):
    from concourse.bass_utils import run_bass_kernel_spmd

    nc = _get_program(tiles)
    res = run_bass_kernel_spmd(nc, in_maps, core_ids=list(range(N_CORES)))
    return [r["yT"] for r in res.results]


def kernel(hidden_states, routing_mask, wg0, wu0, wd0, wg1, wu1, wd1,
           _run=None):
    hidden_states = np.asarray(hidden_states, dtype=np.float32)
    routing_mask = np.asarray(routing_mask)
    B, S, D = hidden_states.shape
    NTOK = B * S
    x = hidden_states.reshape(NTOK, D)
    mask = routing_mask.reshape(NTOK)
    w_np = [(np.asarray(wg0, np.float32), np.asarray(wu0, np.float32),
             np.asarray(wd0, np.float32)),
            (np.asarray(wg1, np.float32), np.asarray(wu1, np.float32),
             np.asarray(wd1, np.float32))]

    idx = [np.nonzero(mask == e)[0] for e in (0, 1)]
    # If an expert slightly exceeds its balanced share (CPE*1024 tokens),
    # compute the small overflow on the host so device cores keep clean
    # 512-wide tiles.  Only do it when it actually shrinks C.
    balanced = CPE * 1024
    spill = [0, 0]
    for e in (0, 1):
        over = len(idx[e]) - balanced
        if 0 < over <= MAX_SPILL:
            spill[e] = over
    maxpc_ns = max((len(idx[0]) + CPE - 1) // CPE,
                   (len(idx[1]) + CPE - 1) // CPE, 1)
    maxpc_sp = max((len(idx[0]) - spill[0] + CPE - 1) // CPE,
                   (len(idx[1]) - spill[1] + CPE - 1) // CPE, 1)
    if 8 * (-(-maxpc_sp // 8)) >= 8 * (-(-maxpc_ns // 8)):
        spill = [0, 0]
    spill_ids = [idx[e][len(idx[e]) - spill[e]:] for e in (0, 1)]
    dev_ids = [idx[e][:len(idx[e]) - spill[e]] for e in (0, 1)]

    maxpc = max((len(dev_ids[0]) + CPE - 1) // CPE,
                (len(dev_ids[1]) + CPE - 1) // CPE, 1)
    C = max(8 * (-(-maxpc // 8)), 32)
    tiles = _token_tiles(C)

    w1_packed = [_pack_w1(w_np[0][0], w_np[0][1]),
                 _pack_w1(w_np[1][0], w_np[1][1])]
    wd_packed = [_pack_wd(w_np[0][2]), _pack_wd(w_np[1][2])]

    in_maps = []
    chunks = []  # (expert, token_indices) per core
    for core in range(N_CORES):
        e = core // CPE
        slot = core % CPE
        ids = dev_ids[e]
        # split ids into CPE nearly-equal chunks
        bounds = [(len(ids) * i) // CPE for i in range(CPE + 1)]
        ids_c = ids[bounds[slot]:bounds[slot + 1]]
        chunks.append((e, ids_c))

        xc = np.zeros((C, D), dtype=np.float32)
        xc[: len(ids_c)] = x[ids_c]
        # xT[p, kd, c] = xc[c, kd*128 + p]
        xT = np.ascontiguousarray(
            xc.reshape(C, KD, P).transpose(2, 1, 0)
        ).astype(BF16)
        in_maps.append({
            "xT": xT,
            "w1": w1_packed[e],
            "wd": wd_packed[e],
        })

    run = _run if _run is not None else _run_device
    outs = run(in_maps, tiles)

    y_full = np.zeros((NTOK, D), dtype=np.float32)
    for core in range(N_CORES):
        _, ids_c = chunks[core]
        if len(ids_c) == 0:
            continue
        yTc = np.asarray(outs[core]).astype(np.float32).reshape(D, C)
        y_full[ids_c] = yTc[:, : len(ids_c)].T
    for e in (0, 1):
        if len(spill_ids[e]):
            wg, wu, wd = w_np[e]
            y_full[spill_ids[e]] = _host_mlp(x[spill_ids[e]], wg, wu, wd)
    return y_full.reshape(B, S, D)


# revision 21
# speedup vs baseline: 1.0049x; 1.0016x over previous
"""MoE (2-expert SwiGLU) Trainium2 kernel, 8-core SPMD.

Strategy: since the MLPs have no biases and silu(0) = 0, MLP(0) = 0, so each
token only needs the expert it is routed to.  The host gathers tokens by
expert (MoE dispatch), cores 0-3 process expert-0 tokens and cores 4-7
expert-1 tokens, each core running a dense SwiGLU MLP with its expert's
weights.  The host scatters per-core outputs back into the full [B, S, D]
output.  If an expert draws slightly more than its 4 cores' balanced share
(4096 tokens), the small overflow (<=64 tokens) is computed on the host so
every core runs the same C<=1024 token capacity with clean 512-wide matmul
tiles.

Device dataflow (per core, transposed so no on-chip transposes are needed):
  yT = Wd^T @ (silu(Wg^T @ xT) * (Wu^T @ xT))
Weights are the stationary matmul operand, token-columns the moving operand.
All matmuls are bf16 with fp32 PSUM accumulation.  The FF intermediate `h`
for all of a core's tokens stays resident in SBUF, so each weight byte is
DMA'd exactly once per core.

Schedule notes (from perfetto/NTFF analysis of the previous version):
  - Two HWDGE rings are used: the sync ring streams the stage-1 weights
    (the critical stream), the scalar ring carries the token activations and
    prefetches ALL stage-2 down-weights into SBUF during stage 1, so the
    stage-1 -> stage-2 transition has no DMA dependency.
  - The PE HAM clock-gate needs ~3.4us of sustained busy time to reach
    2.4 GHz.  A short run of garbage matmuls on a memset scratch tile keeps
    the PE busy (and warms it) while the first real DMAs land.
  - PSUM is fully double-buffered in stage 1 (2 tags x 4 banks), so the PE
    never waits on the Act/DVE drain of the previous iteration.
  - The final output tile is stored in small chunks on alternating rings so
    the post-last-matmul tail is short.
"""

import sys

for _p in ("/opt/trn_rl_repo", "/root/.axon_site/_ro/trn_rl_repo"):
    if _p not in sys.path:
        sys.path.append(_p)

import numpy as np
import ml_dtypes

BF16 = ml_dtypes.bfloat16

D_MODEL = 1024
D_FF = 4096
P = 128
KD = D_MODEL // P  # 8   k-tiles over d_model
MF = D_FF // P     # 32  tiles over d_ff
N_CORES = 8
CPE = 4            # cores per expert
MAX_SPILL = 64     # max tokens/expert computed on host to even out cores

_program_cache: dict[tuple, object] = {}


def _token_tiles(C: int) -> tuple:
    """Split C (a multiple of 8) into the fewest <=512 tiles, near-equal,
    each a multiple of 8."""
    NT = max(1, -(-C // 512))
    t = 8 * (-(-C // (NT * 8)))
    tiles = []
    left = C
    for _ in range(NT):
        s = min(t, left)
        tiles.append(s)
        left -= s
    assert left == 0 and all(0 < s <= 512 and s % 8 == 0 for s in tiles), (C, tiles)
    return tuple(tiles)


def _build_program(tiles: tuple):
    """Bass program for one core: x [D,C] -> y [D,C], C = sum(tiles) tokens."""
    import concourse.tile as tile
    from concourse import mybir, bacc

    C = sum(tiles)
    NT = len(tiles)
    offs = [sum(tiles[:i]) for i in range(NT)]
    TSMAX = max(tiles)
    f32 = mybir.dt.float32
    b16 = mybir.dt.bfloat16

    # Keep the full stage-2 weight set resident in SBUF when it fits next to
    # x/h (it always does for the balanced C<=1024 case).  Very imbalanced
    # routing (huge C) needs the lean streaming layout to fit SBUF.
    wd_resident = C <= 1280
    lean = C > 1536

    nc = bacc.Bacc()
    xT = nc.declare_dram_parameter("xT", [P, KD, C], b16, isOutput=False)
    # w1[mf, p, kd, gu, c] = (wg if gu==0 else wu)[kd*128 + p, mf*128 + c]
    w1 = nc.declare_dram_parameter("w1", [MF, P, KD, 2, P], b16, isOutput=False)
    # wdp[md, p, kf, c] = wd[kf*128 + p, md*128 + c]
    wdp = nc.declare_dram_parameter("wd", [KD, P, MF, P], b16, isOutput=False)
    yT = nc.declare_dram_parameter("yT", [KD, P, C], b16, isOutput=True)

    with tile.TileContext(nc) as tc:
        with (
            tc.tile_pool(name="xp", bufs=1) as xp,
            tc.tile_pool(name="hp", bufs=1) as hp,
            tc.tile_pool(name="wsp", bufs=1) as wsp,
            tc.tile_pool(name="w1p", bufs=(2 if lean else 3)) as w1p,
            tc.tile_pool(name="wdpool", bufs=(1 if wd_resident else 2)) as wdpool,
            tc.tile_pool(name="silp", bufs=(2 if lean else 4)) as silp,
            tc.tile_pool(name="yp", bufs=4) as yp,
        ):
            x_sb = xp.tile([P, KD, C], b16)
            h_sb = hp.tile([P, MF, C], b16)
            ws = wsp.tile([P, 256], b16)
            nc.vector.memset(ws[:], 0.0)

            # Startup staging.  Sync ring: x k-slice 0 interleaved with the
            # first weight tile in kd-growing chunks (the PE consumes
            # kd-major), then the rest of the w1 stream.  Scalar ring: the
            # remaining x k-slices (its first trigger is delayed ~1.3us by
            # the activation-table load).  The stage-2 wd prefetch is spread
            # over mid-stage-1 iterations so it does not steal SDMA
            # bandwidth from the startup-critical x/w1 transfers.
            wt0 = w1p.tile([P, KD, 2, P], b16, tag="wt", name="wt_0")
            nc.sync.dma_start(wt0[:, 0], w1[0, :, 0])
            nc.sync.dma_start(wt0[:, 1:4], w1[0, :, 1:4])
            nc.sync.dma_start(wt0[:, 4:], w1[0, :, 4:])
            for kd in range(KD):
                nc.scalar.dma_start(x_sb[:, kd], xT[:, kd])
            if wd_resident:
                wd_sb = wdpool.tile([P, KD, MF, P], b16, name="wd_all")

            # PSUM: psg/psu/psy tiles span NT banks; matmul t writes the
            # bank-aligned [512*t, 512*t + tiles[t]) slice.  One pool for
            # both stages (a pool boundary would stall stage 2 on the drain
            # of the last stage-1 iteration).
            # When every tile but the last is 512-wide, token offsets equal
            # the bank-aligned PSUM offsets (offs[t] == 512*t), so the
            # silu/mul can run as single instructions over [:, :C].
            PSW = 512 * NT
            uniform = all(s == 512 for s in tiles[:-1]) and not lean
            ps_bufs = max(1, 8 // (2 * NT))
            with tc.tile_pool(name="ps1", bufs=ps_bufs, space="PSUM") as ps1:
                # Garbage matmuls on the memset scratch keep the PE busy (and
                # the HAM clock-gate warming) while the first DMAs land.  The
                # warm tile shares the psg tag, so its bank is recycled (with
                # a WAR dependency) by a later iteration.
                wm = ps1.tile([P, PSW], f32, tag="psg", name="warm")
                for i in range(18):
                    nc.tensor.matmul(
                        wm[:, :256], ws[:, :128], ws[:],
                        start=(i == 0), stop=(i == 17),
                    )
                for mf in range(MF):
                    if mf == 0:
                        wt = wt0
                    else:
                        wt = w1p.tile([P, KD, 2, P], b16, tag="wt",
                                      name=f"wt_{mf}")
                        # w1[1] rides the scalar ring FIFO behind the x
                        # chunks (in two halves, so mf=1's first k-tiles can
                        # start as soon as the first half lands) and doesn't
                        # steal HBM bandwidth from the startup-critical x
                        # stream; later tiles stream on the sync ring,
                        # throttled by the pool buffers.
                        if mf == 1:
                            nc.scalar.dma_start(wt[:, :4], w1[mf, :, :4])
                            nc.scalar.dma_start(wt[:, 4:], w1[mf, :, 4:])
                        else:
                            nc.sync.dma_start(wt[:], w1[mf])
                    if wd_resident and mf % 2 == 0 and 10 <= mf < 10 + 2 * KD:
                        nc.scalar.dma_start(wd_sb[:, (mf - 10) // 2],
                                            wdp[(mf - 10) // 2])
                    psg = ps1.tile([P, PSW], f32, tag="psg", name=f"psg_{mf}")
                    psu = ps1.tile([P, PSW], f32, tag="psu", name=f"psu_{mf}")
                    for kd in range(KD):
                        for gu in range(2):
                            ps = psg if gu == 0 else psu
                            for t in range(NT):
                                nc.tensor.matmul(
                                    ps[:, 512 * t:512 * t + tiles[t]],
                                    wt[:, kd, gu],
                                    x_sb[:, kd, offs[t]:offs[t] + tiles[t]],
                                    start=(kd == 0),
                                    stop=(kd == KD - 1),
                                )
                    if uniform:
                        sil = silp.tile([P, PSW], f32, tag="sil",
                                        name=f"sil_{mf}")
                        nc.scalar.activation(
                            sil[:, :C], psg[:, :C],
                            mybir.ActivationFunctionType.Silu,
                        )
                        nc.vector.tensor_mul(h_sb[:, mf], sil[:, :C],
                                             psu[:, :C])
                    else:
                        for t in range(NT):
                            sil = silp.tile([P, TSMAX], f32, tag="sil",
                                            name=f"sil_{mf}_{t}")
                            nc.scalar.activation(
                                sil[:, :tiles[t]],
                                psg[:, 512 * t:512 * t + tiles[t]],
                                mybir.ActivationFunctionType.Silu,
                            )
                            nc.vector.tensor_mul(
                                h_sb[:, mf, offs[t]:offs[t] + tiles[t]],
                                sil[:, :tiles[t]],
                                psu[:, 512 * t:512 * t + tiles[t]],
                            )

                # Stage 2: y = Wd^T h, laid out [d-part, C].  psy reuses the
                # psg tag so there is no pool boundary between the stages.
                for md in range(KD):
                    if wd_resident:
                        wdt = wd_sb[:, md]
                    else:
                        wdtile = wdpool.tile([P, MF, P], b16, tag="wds",
                                             name=f"wd_{md}")
                        nc.sync.dma_start(wdtile[:], wdp[md])
                        wdt = wdtile[:]
                    psy = ps1.tile([P, PSW], f32, tag="psg", name=f"psy_{md}")
                    for t in range(NT):
                        sz = tiles[t]
                        off = offs[t]
                        last = (md == KD - 1 and t == NT - 1)
                        if not last:
                            for kf in range(MF):
                                nc.tensor.matmul(
                                    psy[:, 512 * t:512 * t + sz],
                                    wdt[:, kf],
                                    h_sb[:, kf, off:off + sz],
                                    start=(kf == 0),
                                    stop=(kf == MF - 1),
                                )
                            # Copies run on the scalar engine: it is idle in
                            # stage 2, and the strict-FIFO DVE queue would put
                            # these in the PE's semaphore dependency path.
                            y_sb = yp.tile([P, TSMAX], b16, tag="y",
                                           name=f"y_{md}_{t}")
                            nc.scalar.copy(
                                y_sb[:, :sz], psy[:, 512 * t:512 * t + sz])
                            nc.sync.dma_start(yT[md, :, off:off + sz],
                                              y_sb[:, :sz])
                        else:
                            # The very last tile runs as two accumulation
                            # groups in different PSUM banks, so the first
                            # group's store overlaps the second's matmuls and
                            # the post-last-matmul tail is one short (128
                            # token) copy + DMA.
                            h1 = sz - 128 if sz >= 256 else (sz // 2) // 8 * 8
                            psyb = ps1.tile([P, PSW], f32, tag="psu",
                                            name="psy_last")
                            for g, (a, b, pst, po) in enumerate(
                                    [(0, h1, psy, 512 * t),
                                     (h1, sz, psyb, 0)]):
                                for kf in range(MF):
                                    nc.tensor.matmul(
                                        pst[:, po + a:po + b],
                                        wdt[:, kf],
                                        h_sb[:, kf, off + a:off + b],
                                        start=(kf == 0),
                                        stop=(kf == MF - 1),
                                    )
                                y_sb = yp.tile([P, TSMAX], b16, tag="y",
                                               name=f"y_last_{g}")
                                nc.scalar.copy(y_sb[:, a:b],
                                               pst[:, po + a:po + b])
                                nc.sync.dma_start(yT[md, :, off + a:off + b],
                                                  y_sb[:, a:b])

    nc.compile()
    return nc


def _get_program(tiles: tuple):
    if tiles not in _program_cache:
        _program_cache[tiles] = _build_program(tiles)
    return _program_cache[tiles]


def _pack_w1(wg: np.ndarray, wu: np.ndarray) -> np.ndarray:
    """[D, F] x2 -> [MF, P, KD, 2, P] bf16, matching the kernel's layout."""
    # w1[mf, p, kd, gu, c] = w_gu[kd*128 + p, mf*128 + c]
    stack = np.stack([wg, wu], axis=0)            # [gu, D, F]
    r = stack.reshape(2, KD, P, MF, P)            # [gu, kd, p, mf, c]
    return np.ascontiguousarray(r.transpose(3, 2, 1, 0, 4)).astype(BF16)


def _pack_wd(wd: np.ndarray) -> np.ndarray:
    """[F, D] -> [KD, P, MF, P] bf16. wdp[md, p, kf, c] = wd[kf*128+p, md*128+c]"""
    r = wd.reshape(MF, P, KD, P)                  # [kf, p, md, c]
    return np.ascontiguousarray(r.transpose(2, 1, 0, 3)).astype(BF16)


def _host_mlp(x, wg, wu, wd):
    g = x @ wg
    u = x @ wu
    h = (g / (1.0 + np.exp(-g))) * u
    return h @ wd


def _run_device(in_maps, tiles):
    from concourse.bass_utils import run_bass_kernel_spmd

    nc = _get_program(tiles)
    res = run_bass_kernel_spmd(nc, in_maps, core_ids=list(range(N_CORES)))
    return [r["yT"] for r in res.results]


def kernel(hidden_states, routing_mask, wg0, wu0, wd0, wg1, wu1, wd1,
           _run=None):
    hidden_states = np.asarray(hidden_states, dtype=np.float32)
    routing_mask = np.asarray(routing_mask)
    B, S, D = hidden_states.shape
    NTOK = B * S
    x = hidden_states.reshape(NTOK, D)
    mask = routing_mask.reshape(NTOK)
    w_np = [(np.asarray(wg0, np.float32), np.asarray(wu0, np.float32),
             np.asarray(wd0, np.float32)),
            (np.asarray(wg1, np.float32), np.asarray(wu1, np.float32),
             np.asarray(wd1, np.float32))]

    idx = [np.nonzero(mask == e)[0] for e in (0, 1)]
    # Device capacity C (tokens/core): the smallest multiple of 8 such that
    # the host absorbs at most MAX_SPILL remainder tokens per expert.  Every
    # 8 tokens of C is ~2.6us of device time (768 weight-tile passes).
    maxpc = max((max(0, len(idx[e]) - MAX_SPILL) + CPE - 1) // CPE
                for e in (0, 1))
    C = max(8 * (-(-maxpc // 8)), 32)
    spill = [max(0, len(idx[e]) - CPE * C) for e in (0, 1)]
    spill_ids = [idx[e][len(idx[e]) - spill[e]:] for e in (0, 1)]
    dev_ids = [idx[e][:len(idx[e]) - spill[e]] for e in (0, 1)]
    tiles = _token_tiles(C)

    w1_packed = [_pack_w1(w_np[0][0], w_np[0][1]),
                 _pack_w1(w_np[1][0], w_np[1][1])]
    wd_packed = [_pack_wd(w_np[0][2]), _pack_wd(w_np[1][2])]

    in_maps = []
    chunks = []  # (expert, token_indices) per core
    for core in range(N_CORES):
        e = core // CPE
        slot = core % CPE
        ids = dev_ids[e]
        # split ids into CPE nearly-equal chunks
        bounds = [(len(ids) * i) // CPE for i in range(CPE + 1)]
        ids_c = ids[bounds[slot]:bounds[slot + 1]]
        chunks.append((e, ids_c))

        xc = np.zeros((C, D), dtype=np.float32)
        xc[: len(ids_c)] = x[ids_c]
        # xT[p, kd, c] = xc[c, kd*128 + p]
        xT = np.ascontiguousarray(
            xc.reshape(C, KD, P).transpose(2, 1, 0)
        ).astype(BF16)
        in_maps.append({
            "xT": xT,
            "w1": w1_packed[e],
            "wd": wd_packed[e],
        })

    run = _run if _run is not None else _run_device
    outs = run(in_maps, tiles)

    y_full = np.zeros((NTOK, D), dtype=np.float32)
    for core in range(N_CORES):
        _, ids_c = chunks[core]
        if len(ids_c) == 0:
            continue
        yTc = np.asarray(outs[core]).astype(np.float32).reshape(D, C)
        y_full[ids_c] = yTc[:, : len(ids_c)].T
    for e in (0, 1):
        if len(spill_ids[e]):
            wg, wu, wd = w_np[e]
            y_full[spill_ids[e]] = _host_mlp(x[spill_ids[e]], wg, wu, wd)
    return y_full.reshape(B, S, D)
